# revision 16
# baseline (speedup 1.0000x reference)
"""Trainium2 Bass kernel for nn_BasicQNN: 4-qubit QNN expectation value.

Math: the circuit is  |psi(x)> = U(weights) . (RY(x0)xRY(x1)xRY(x2)xRY(x3)) |0000>
and  y = <psi| Z_0 |psi>.  Since the encoding state is a real product state,
y(x) = sum_{g in {I,Z,X}^4} C_g * prod_i m_i(g_i)   with  m_i = (1, cos x_i, sin x_i)
and C_g = (1/16) <Re(U^+ Z0 U), g0 x g1 x g2 x g3>  computed on host from the
24 weights.  The device kernel evaluates this 81-term multilinear polynomial
per sample with ScalarE Sin activations and a 4-level Horner scheme on VectorE.
"""

import math
import sys

import numpy as np

sys.path.insert(0, "/opt/trn_rl_repo")

NQ = 4
NL = 2
BATCH = 1048576
N_CORES = 8
SHARD = BATCH // N_CORES          # 131072 samples per core
P = 128                           # partitions
PLANE = SHARD // P                # 1024 free elements per partition
FC = 1024                         # free-dim chunk per tile
NT = PLANE // FC                  # tiles per core
ZTOL = 1e-9


# ---------------------------------------------------------------- host math
def _compute_coeffs(weights: np.ndarray) -> np.ndarray:
    """C[3,3,3,3] over basis (I, Z, X) per wire; fp64."""
    w = np.asarray(weights, dtype=np.float64).reshape(NL, NQ, 3)

    def ry(t):
        c, s = np.cos(t / 2), np.sin(t / 2)
        return np.array([[c, -s], [s, c]], dtype=complex)

    def rx(t):
        c, s = np.cos(t / 2), np.sin(t / 2)
        return np.array([[c, -1j * s], [-1j * s, c]], dtype=complex)

    def rz(t):
        return np.array([[np.exp(-1j * t / 2), 0], [0, np.exp(1j * t / 2)]],
                        dtype=complex)

    def on_wire(g, wire):
        out = np.array([[1.0 + 0j]])
        for i in range(NQ):
            out = np.kron(out, g if i == wire else np.eye(2))
        return out

    def cnot(c, t):
        U = np.zeros((16, 16), dtype=complex)
        for k in range(16):
            bits = [(k >> (3 - i)) & 1 for i in range(4)]
            if bits[c] == 1:
                bits[t] ^= 1
            j = sum(b << (3 - i) for i, b in enumerate(bits))
            U[j, k] = 1
        return U

    U = np.eye(16, dtype=complex)
    for layer in range(NL):
        for i in range(NQ):
            U = on_wire(rx(w[layer, i, 0]), i) @ U
            U = on_wire(ry(w[layer, i, 1]), i) @ U
            U = on_wire(rz(w[layer, i, 2]), i) @ U
        for i in range(NQ - 1):
            U = cnot(i, i + 1) @ U
        U = cnot(NQ - 1, 0) @ U

    Z0 = on_wire(np.diag([1.0, -1.0]), 0)
    A = (U.conj().T @ Z0 @ U).real

    I2, Zm, Xm = np.eye(2), np.diag([1.0, -1.0]), np.array([[0.0, 1.0], [1.0, 0.0]])
    ms = [I2, Zm, Xm]
    C = np.zeros((3, 3, 3, 3))
    for a in range(3):
        for b in range(3):
            for c in range(3):
                for d in range(3):
                    Pm = np.kron(np.kron(np.kron(ms[a], ms[b]), ms[c]), ms[d])
                    C[a, b, c, d] = np.sum(A * Pm) / 16.0
    return C


def reference_poly(x: np.ndarray, C: np.ndarray) -> np.ndarray:
    """Host-side evaluation of the same polynomial (for debugging)."""
    m = np.stack([np.ones_like(x), np.cos(x), np.sin(x)], axis=-1)  # [B,4,3]
    return np.einsum("abcd,na,nb,nc,nd->n", C,
                     m[:, 0], m[:, 1], m[:, 2], m[:, 3]).astype(np.float32)


# ---------------------------------------------------------------- bass kernel
_PATCHED = []


def _patch_drain_split():
    """walrus on this toolchain encodes at most one sync-wait per SP CTRL
    instruction; Tile's kernel-tail drain carries one wait per live
    semaphore.  Split them across single-wait NOPs (SP executes in order,
    so the semantics are unchanged)."""
    if _PATCHED:
        return
    import concourse.tile as tile_mod
    import concourse.mybir as _mybir
    from concourse.vector_clock import ScopedClock

    def _dab(self, tick_clock, wait_clock):
        probe = self.nc.sync.nop()
        wait_clock.add_sem_waits(
            probe.ins, ScopedClock({None: tick_clock.global_clock}))
        si = probe.ins.sync_info
        waits = list(si.on_wait) if si is not None else []
        if si is not None:
            si.on_wait = waits[:1]
        for w in waits[1:]:
            extra = self.nc.sync.nop()
            extra.ins.sync_info = _mybir.SyncInfo(on_wait=[w], on_update=[])
        self.nc.sync.drain()
        self.nc.all_engine_barrier()
        assert self.sems is not None
        popped = self.nc._tile_sem_poison_stack.pop()
        assert popped is self._sem_poison
        self.nc.clear_and_free_semaphores(
            list(self.sems.allocated().values()))
        self.nc.all_engine_barrier()

    tile_mod.TileContext._drain_and_barrier = _dab
    _PATCHED.append(True)


def _build_program(C: np.ndarray):
    from concourse import bass
    import concourse.mybir as mybir
    from concourse.tile import TileContext

    _patch_drain_split()

    f32 = mybir.dt.float32
    Act = mybir.ActivationFunctionType
    Op = mybir.AluOpType

    nc = bass.Bass()
    x_ext = nc.declare_dram_parameter("x", [SHARD, 4], f32, isOutput=False)
    y_ext = nc.declare_dram_parameter("y", [SHARD], f32, isOutput=True)

    x_r = x_ext.rearrange("(p n) w -> p (n w)", p=P)      # [128, PLANE*4]
    y_r = y_ext.rearrange("(p n) -> p n", p=P)            # [128, PLANE]

    HALF_PI = math.pi / 2.0

    with TileContext(nc) as tc:
        with tc.tile_pool(name="io", bufs=1) as io_pool, \
             tc.tile_pool(name="trig", bufs=1) as trig_pool, \
             tc.tile_pool(name="work", bufs=1) as work_pool:

            for t in range(NT):
                xt = io_pool.tile([P, FC * 4], f32, name="xt", tag="xt")
                nc.sync.dma_start(
                    out=xt, in_=x_r[:, t * FC * 4:(t + 1) * FC * 4])
                # range-reduce to fractional turns: f = x/2pi - round(x/2pi)
                # in [-0.5, 0.5]; Sin activation then uses scale=2pi (its
                # spline is only valid on [-pi, pi]).
                MAGIC = 1.5 * 2.0 ** 23
                fz = io_pool.tile([P, FC * 4], f32, name="fz", tag="fz")
                gz = io_pool.tile([P, FC * 4], f32, name="gz", tag="gz")
                fk = io_pool.tile([P, FC * 4], f32, name="fk", tag="fk")
                nc.vector.tensor_scalar_mul(out=fz, in0=xt,
                                            scalar1=1.0 / (2.0 * math.pi))
                nc.vector.tensor_scalar(out=gz, in0=fz, scalar1=0.25,
                                        scalar2=None, op0=Op.add)
                nc.vector.tensor_scalar(out=fk, in0=fz, scalar1=MAGIC,
                                        scalar2=MAGIC, op0=Op.add,
                                        op1=Op.subtract)
                nc.vector.tensor_sub(out=fz, in0=fz, in1=fk)
                nc.vector.tensor_scalar(out=fk, in0=gz, scalar1=MAGIC,
                                        scalar2=MAGIC, op0=Op.add,
                                        op1=Op.subtract)
                nc.vector.tensor_sub(out=gz, in0=gz, in1=fk)
                xv = fz.rearrange("p (n w) -> p n w", w=4)    # sin source
                xpv = gz.rearrange("p (n w) -> p n w", w=4)   # cos source

                # trig tiles: cos/sin of each wire's angle
                trig = {}
                for i in range(NQ):
                    ci = trig_pool.tile([P, FC], f32, name=f"ct{i}", tag=f"c{i}")
                    si = trig_pool.tile([P, FC], f32, name=f"st{i}", tag=f"s{i}")
                    nc.scalar.activation(out=ci, in_=xpv[:, :, i], func=Act.Sin,
                                         bias=0.0, scale=2.0 * math.pi)
                    nc.scalar.activation(out=si, in_=xv[:, :, i],
                                                  func=Act.Sin,
                                                  bias=0.0,
                                                  scale=2.0 * math.pi)
                    trig[(i, "c")] = ci
                    trig[(i, "s")] = si

                c3, s3 = trig[(3, "c")], trig[(3, "s")]
                c2, s2 = trig[(2, "c")], trig[(2, "s")]
                c1, s1 = trig[(1, "c")], trig[(1, "s")]
                c0, s0 = trig[(0, "c")], trig[(0, "s")]

                # fixed per-iteration work buffers, written in place
                wbuf = {}
                for tag in ("t0", "t1", "t2", "sb0", "sb1", "sb2",
                            "ra0", "ra1", "ra2", "tmp"):
                    wbuf[tag] = work_pool.tile([P, FC], f32, name=tag, tag=tag)

                def nz(v):
                    return abs(v) > ZTOL

                # node := ('z',), ('k', const), ('t', AP)
                def eval_triple(dst_tag, nI, nZ, nX, cf, sf, dst_ap=None):
                    """Return node for nI + cf*nZ + sf*nX, written into the
                    named work buffer (in place)."""
                    const_p = nI[1] if nI[0] == "k" else 0.0
                    prods = [(f, nd) for f, nd in ((cf, nZ), (sf, nX))
                             if nd[0] != "z"]
                    if not prods and nI[0] != "t":
                        return ("k", const_p) if nz(const_p) else ("z",)
                    dst = dst_ap if dst_ap is not None else wbuf[dst_tag]
                    init = False
                    # const-coefficient products first (fuse pending const)
                    for f, nd in prods:
                        if nd[0] != "k":
                            continue
                        v = float(nd[1])
                        if not init:
                            if nz(const_p):
                                nc.vector.tensor_scalar(
                                    out=dst, in0=f, scalar1=v,
                                    scalar2=float(const_p),
                                    op0=Op.mult, op1=Op.add)
                                const_p = 0.0
                            else:
                                nc.vector.tensor_scalar_mul(out=dst, in0=f,
                                                            scalar1=v)
                            init = True
                        else:
                            tmp = wbuf["tmp"]
                            nc.vector.tensor_scalar_mul(out=tmp, in0=f,
                                                        scalar1=v)
                            nc.vector.tensor_add(out=dst, in0=dst, in1=tmp)
                    # tile-valued products
                    for f, nd in prods:
                        if nd[0] != "t":
                            continue
                        if not init:
                            nc.vector.tensor_mul(out=dst, in0=f, in1=nd[1])
                            init = True
                        else:
                            tmp = wbuf["tmp"]
                            nc.vector.tensor_mul(out=tmp, in0=f, in1=nd[1])
                            nc.vector.tensor_add(out=dst, in0=dst, in1=tmp)
                    if nI[0] == "t":
                        if init:
                            nc.vector.tensor_add(out=dst, in0=dst, in1=nI[1])
                        else:
                            nc.vector.tensor_copy(out=dst, in_=nI[1])
                        init = True
                    if nz(const_p) and init:
                        nc.vector.tensor_scalar_add(out=dst, in0=dst,
                                                    scalar1=float(const_p))
                    return ("t", dst)

                def knode(v):
                    return ("k", float(v)) if nz(v) else ("z",)

                Rn = []
                for a in range(3):
                    Sn = []
                    for b in range(3):
                        Tn = [eval_triple(f"t{g2}",
                                          knode(C[a, b, g2, 0]),
                                          knode(C[a, b, g2, 1]),
                                          knode(C[a, b, g2, 2]),
                                          c3, s3)
                              for g2 in range(3)]
                        Sn.append(eval_triple(f"sb{b}", Tn[0], Tn[1], Tn[2],
                                              c2, s2))
                    Rn.append(eval_triple(f"ra{a}", Sn[0], Sn[1], Sn[2],
                                          c1, s1))
                yt = io_pool.tile([P, FC], f32, name="yt", tag="yt")
                yn = eval_triple("yy", Rn[0], Rn[1], Rn[2], c0, s0, dst_ap=yt)
                if yn[0] != "t":
                    nc.vector.memset(yt, float(yn[1]) if yn[0] == "k" else 0.0)
                nc.sync.dma_start(out=y_r[:, t * FC:(t + 1) * FC], in_=yt)

    return nc


# ---------------------------------------------------------------- entry point
_CACHE = {}


def kernel(x: np.ndarray, weights: np.ndarray) -> np.ndarray:
    from concourse.bass_utils import run_bass_kernel_spmd

    x = np.ascontiguousarray(np.asarray(x, dtype=np.float32))
    C = _compute_coeffs(weights)

    key = hash(C.tobytes())
    if key not in _CACHE:
        _CACHE[key] = _build_program(C)
    nc = _CACHE[key]

    shards = x.reshape(N_CORES, SHARD, 4)
    in_maps = [{"x": shards[i]} for i in range(N_CORES)]
    res = run_bass_kernel_spmd(nc, in_maps, list(range(N_CORES)))
    y = np.concatenate([np.asarray(r["y"]).reshape(SHARD) for r in res.results])
    return y.astype(np.float32)


if __name__ == "__main__":
    rng = np.random.default_rng(0)
    x = rng.normal(size=(BATCH, NQ)).astype(np.float32)
    w = rng.normal(size=(NL * NQ * 3,)).astype(np.float32)
    y = kernel(x, w)
    print("y", y.shape, y.dtype, y[:8])
    print("host poly", reference_poly(x[:8], _compute_coeffs(w)))
